# revision 27
# baseline (speedup 1.0000x reference)
"""MultiHeadAttention Trainium2 kernel (8 NeuronCores).

Sharding: 8 cores = 4 batches x 2 head-groups (8 heads each).
Core g: batch b = g//2, head-group hg = g%2 (heads hg*8 .. hg*8+7).

Device program (identical on all cores, SPMD):
  inputs (per core): xq/xk/xv = x[b].T [1024, 2048] bf16,
    wq/wk/wv = w[:, hg*512:(hg+1)*512] as [128, 8, 512] bf16,
    wo = [128, 4, 1024] bf16, bq [128, 4] f32.
  output: yt [1024, 2048] = (partial out).T, host adds b_v @ w_o + b_o.

Math identities (exact in real arithmetic):
  softmax((Q+bq)(K+bk)^T) == softmax((Q+bq) K^T)   [k-constant terms cancel]
  attn @ (V + bv) @ Wo + bo == attn @ V @ Wo + (bv @ Wo + bo)  [rows sum to 1]
  exp without max-subtraction: scores reach ~8.4; e^8.4 = 4447 fits fp16.

Dtypes: all matmul paths bf16/fp16 (fp8 anywhere on the logit or V path
measures 5-7% max-rel-err -- peaked attention rows amplify quantization;
see sims). Scores fp16 row-tile pairs at (0,0)/(64,0) (true 2x
co-stream). Attention weights fp16: ACT exp for chunk-groups not in
DVE_G; DVE_G groups use a DVE fast-exp writing fp16 BITS via
u16 = round(raw*184.6649 + 15315.0) (Schraudolph-style, |rel err| <=
~3% sawtooth, <1e-3 end-to-end), bitcast to fp16 for the AV matmul --
offloads that fraction of the exp wall off the Scalar engine. V tiles
fp16 with an exact fp16 ones-column producing Z at psum row 64.
Out-proj bf16. psum f32.
"""
import numpy as np

B, S, D = 4, 2048, 1024
HPC, PAIRS, QB, KC, CC2, CC = 8, 4, 4, 16, 4, 8
N = 512
DVE_G = (6, 7)          # chunk-groups whose exp runs on DVE (fast-exp)
FE_MUL = 184.66496      # 1024*log2(e)*0.125
FE_ADD = 15315.0        # 1024*15 - 45 (Schraudolph corr)

_CACHE = {}


def _build():
    from concourse import bacc
    import concourse.mybir as mybir
    import concourse.tile as tile

    F32 = mybir.dt.float32
    BF16 = mybir.dt.bfloat16
    FP16 = mybir.dt.float16
    F8 = mybir.dt.float8e4
    U16 = mybir.dt.uint16
    AF = mybir.ActivationFunctionType
    DR = mybir.MatmulPerfMode.DoubleRow
    MUL = mybir.AluOpType.mult
    ADD = mybir.AluOpType.add

    nc = bacc.Bacc()
    xq_d = nc.declare_dram_parameter("xq", [D, S], BF16, isOutput=False)
    xk_d = nc.declare_dram_parameter("xk", [D, S], BF16, isOutput=False)
    xv_d = nc.declare_dram_parameter("xv", [D, S], BF16, isOutput=False)
    wq_d = nc.declare_dram_parameter("wq", [128, CC, N], BF16, isOutput=False)
    wk_d = nc.declare_dram_parameter("wk", [128, CC, N], BF16, isOutput=False)
    wv_d = nc.declare_dram_parameter("wv", [128, CC, N], BF16, isOutput=False)
    wo_d = nc.declare_dram_parameter("wo", [128, PAIRS, D], BF16, isOutput=False)
    bq_d = nc.declare_dram_parameter("bq", [128, PAIRS], F32, isOutput=False)
    yt_d = nc.dram_tensor("yt", [D, S], F32, kind="ExternalOutput")

    with tile.TileContext(nc) as tc:
        with (
            tc.tile_pool(name="per", bufs=1) as per,
            tc.tile_pool(name="wp", bufs=1) as wp,
            tc.tile_pool(name="xs", bufs=1) as xsp,
            tc.tile_pool(name="ep", bufs=1) as epp,
            tc.tile_pool(name="msc", bufs=1) as msc,
            tc.tile_pool(name="pp", bufs=1, space="PSUM") as pp,
        ):
            # ---- persistent tiles ----
            kh = [per.tile([128, S], FP16, name=f"kh{p}", tag="kh", bufs=PAIRS)
                  for p in range(PAIRS)]
            qh = [per.tile([128, S], FP16, name=f"qh{p}", tag="qh", bufs=PAIRS)
                  for p in range(PAIRS)]
            # V per 2-chunk group: [k-part, chunk-in-pair, head, dv|ones] fp16
            vs = [per.tile([128, 2, HPC, 65], FP16, name=f"vs{g}", tag="vs",
                           bufs=KC // 2) for g in range(KC // 2)]
            # ---- weights (tag-shared slots; wo reuses a freed slot later) ----
            wk_s = wp.tile([128, CC, N], BF16, name="wk_s", tag="w2", bufs=2)
            wq_s = wp.tile([128, CC, N], BF16, name="wq_s", tag="w2", bufs=2)
            wv_s = wp.tile([128, CC, N], BF16, name="wv_s", tag="wv", bufs=1)
            for c in range(CC):
                nc.scalar.dma_start(out=wk_s[:, c, :], in_=wk_d[:, c, :])
            nc.scalar.dma_start(out=wv_s, in_=wv_d[:, :, :])
            nc.scalar.dma_start(out=wq_s, in_=wq_d[:, :, :])
            bqt = per.tile([128, PAIRS], F32, name="bqt", tag="bqt", bufs=1)
            nc.scalar.dma_start(out=bqt, in_=bq_d[:, :])

            # PSUM pools: "big" 2x[128,1024]f32 (scores + oproj), "sm"
            # 4x[128,512]f32 (phase-1 proj psums, then AV po0/po1).

            # ================= phase 1: projections =================
            def emit_x_dmas(x_d, j, tag, name, deferred=False):
                xt = [xsp.tile([128, N], BF16, name=f"{name}{c}", tag=tag,
                               bufs=24) for c in range(CC)]
                for c in range(CC):
                    # Deferred sets issue from the Scalar queue: gpsimd/sync
                    # carry dependent norm/output work by then, and their
                    # head-of-line waits delay the DMA config by tens of us.
                    if deferred:
                        eng = nc.scalar
                    else:
                        eng = nc.sync if c % 2 == 0 else nc.gpsimd
                    eng.dma_start(
                        out=xt[c],
                        in_=x_d[128 * c:128 * (c + 1), N * j:N * (j + 1)])
                return xt

            def emit_kq_dmas(x_d, j, tag, name, deferred=False):
                return emit_x_dmas(x_d, j, tag, name, deferred)

            def emit_v_dmas(j, deferred=False):
                return emit_x_dmas(xv_d, j, "xv", "xvt", deferred)

            def emit_kq_unit(w_s, dst, biased, j, p, xt, tag):
                ps = pp.tile([128, N], F32, name="ps", tag=tag,
                             bufs=4 if tag == "sm" else 2)
                for c in range(CC):
                    nc.tensor.matmul(ps, w_s[:, c, 128 * p:128 * (p + 1)],
                                     xt[c], start=(c == 0), stop=(c == CC - 1))
                if biased:
                    nc.vector.tensor_scalar_add(
                        dst[p][:, N * j:N * (j + 1)], ps, bqt[:, p:p + 1])
                else:
                    nc.vector.tensor_copy(dst[p][:, N * j:N * (j + 1)], ps)

            def emit_v_unit(q4, t2, xvt, tag):
                t = 4 * q4 + t2
                g, tcg = t // 2, t % 2
                ps = pp.tile([128, N], F32, name="psv", tag=tag,
                             bufs=4 if tag == "sm" else 2)
                for c in range(CC):
                    nc.tensor.matmul(ps, xvt[c][:, 128 * t2:128 * (t2 + 1)],
                                     wv_s[:, c, :], start=(c == 0),
                                     stop=(c == CC - 1))
                if tcg == 0:
                    nc.gpsimd.memset(vs[g][:, :, :, 64:65], 1.0)
                nc.vector.tensor_copy(
                    vs[g][:, tcg, :, 0:64],
                    ps.rearrange("p (h e) -> p h e", e=64))

            for j in range(QB):
                xt = emit_kq_dmas(xk_d, j, "xs", "xkt")
                for p in range(PAIRS):
                    emit_kq_unit(wk_s, kh, False, j, p, xt, "sm")
            xt = emit_kq_dmas(xq_d, 0, "xs", "xqt")
            for p in range(PAIRS):
                emit_kq_unit(wq_s, qh, True, 0, p, xt, "sm")
            xvt0 = emit_v_dmas(0)
            for t2 in range(4):
                emit_v_unit(0, t2, xvt0, "sm")

            # wo loaded into a freed w2 slot
            wo_s = wp.tile([128, PAIRS, D], BF16, name="wo_s", tag="w2", bufs=2)
            nc.scalar.dma_start(out=wo_s, in_=wo_d[:, :, :])

            # deferred phase-1 work, interleaved into stream steps
            xsets = {}
            side = {}

            def at(step, f):
                side.setdefault(step, []).append(f)

            def mk_vdma(q4):
                return lambda: xsets.__setitem__(
                    ("v", q4), emit_v_dmas(q4, deferred=True))

            def mk_vunit(q4, t2):
                return lambda: emit_v_unit(q4, t2, xsets[("v", q4)], "big")

            def mk_qdma(j):
                return lambda: xsets.__setitem__(
                    ("q", j), emit_kq_dmas(xq_d, j, "xs", "xqt",
                                           deferred=True))

            def mk_qunit(j, p):
                return lambda: emit_kq_unit(wq_s, qh, True, j, p,
                                            xsets[("q", j)], "big")

            xsets[("v", 1)] = emit_v_dmas(1)
            at(0, mk_vdma(2))
            at(1, mk_qdma(1))
            at(2, mk_vdma(3))
            at(10, mk_qdma(2))
            at(24, mk_qdma(3))
            for q4 in (1, 2, 3):
                for t2 in range(4):
                    at((q4 - 1) * 4 + t2, mk_vunit(q4, t2))
            qsched = {1: (12, 15, 18, 21), 2: (26, 31, 36, 41),
                      3: (48, 56, 64, 72)}
            for jq in (1, 2, 3):
                for p in range(PAIRS):
                    at(qsched[jq][p], mk_qunit(jq, p))

            # ================= phase 2: attention + out-proj =================
            NG = KC // 2
            stream = [(j, p, g) for j in range(QB) for p in range(PAIRS)
                      for g in range(NG)]
            ctx = {}     # (j, p) -> dict(po0, po1, e[g])
            ots = {}     # j -> [ot tiles]
            oproj_pending = []

            def emit_scores_exp(j, p, g):
                if g == 0:
                    ctx[(j, p)] = {
                        "po0": pp.tile([65, N], F32, name="po0", tag="sm", bufs=4),
                        "po1": pp.tile([65, N], F32, name="po1", tag="sm", bufs=4),
                        "e": [None] * NG,
                    }
                st_ = ctx[(j, p)]
                if g in DVE_G:
                    ecu = epp.tile([128, 2, 2 * N], U16, name="ecu", tag="ep",
                                   bufs=12)
                    ec = ecu.bitcast(FP16)
                else:
                    ecu = None
                    ec = epp.tile([128, 2, 2 * N], FP16, name="ec", tag="ep",
                                  bufs=12)
                for ci in range(2):
                    c = 2 * g + ci
                    sc = pp.tile([128, 2 * N], F32, name="sc", tag="big", bufs=2)
                    nc.tensor.matmul(
                        sc[:, 0:N],
                        kh[p][0:64, 128 * c:128 * (c + 1)],
                        qh[p][0:64, N * j:N * (j + 1)],
                        start=True, stop=True, tile_position=(0, 0))
                    nc.tensor.matmul(
                        sc[:, N:2 * N],
                        kh[p][64:128, 128 * c:128 * (c + 1)],
                        qh[p][64:128, N * j:N * (j + 1)],
                        start=True, stop=True, tile_position=(64, 0))
                    if g in DVE_G:
                        # fast-exp: fp16 bits u16 = round(raw*a + b)
                        nc.vector.tensor_scalar(ecu[:, ci, :], sc,
                                                FE_MUL, FE_ADD, MUL, ADD)
                    else:
                        nc.scalar.activation(ec[:, ci, :], sc, AF.Exp,
                                             scale=0.125)
                st_["e"][g] = ec

            def emit_av(j, p, g):
                st_ = ctx[(j, p)]
                ec = st_["e"][g]
                for ci in range(2):
                    c = 2 * g + ci
                    ss, se = (c == 0), (c == KC - 1)
                    nc.tensor.matmul(st_["po0"], vs[g][:, ci, 2 * p, :],
                                     ec[:, ci, 0:N], start=ss, stop=se)
                    nc.tensor.matmul(st_["po1"], vs[g][:, ci, 2 * p + 1, :],
                                     ec[:, ci, N:2 * N], start=ss, stop=se)

            def emit_norm(j, p):
                st_ = ctx.pop((j, p))
                if j not in ots:
                    ots[j] = [epp.tile([128, N], BF16, name=f"ot{q}", tag="ot",
                                       bufs=8) for q in range(PAIRS)]
                ot = ots[j]
                # Drain psum to SBUF immediately (fast DVE ops) so the po
                # slots recycle without waiting on the recip/broadcast chain.
                zrow = msc.tile([1, 2 * N], F32, name="zrow", tag="zrow", bufs=1)
                raw0 = msc.tile([64, N], F32, name="raw0", tag="raw0", bufs=2)
                raw1 = msc.tile([64, N], F32, name="raw1", tag="raw1", bufs=2)
                nc.vector.tensor_copy(zrow[:, 0:N], st_["po0"][64:65, :])
                nc.vector.tensor_copy(zrow[:, N:2 * N], st_["po1"][64:65, :])
                nc.vector.tensor_copy(raw0, st_["po0"][0:64, :])
                nc.vector.tensor_copy(raw1, st_["po1"][0:64, :])
                rz = msc.tile([1, 2 * N], F32, name="rz", tag="rz", bufs=1)
                nc.vector.reciprocal_approx_fast(rz, zrow)
                rbcA = msc.tile([64, N], F32, name="rbcA", tag="rbcA", bufs=1)
                rbcB = msc.tile([64, N], F32, name="rbcB", tag="rbcB", bufs=1)
                nc.gpsimd.partition_broadcast(rbcA, rz[0:1, 0:N])
                nc.gpsimd.partition_broadcast(rbcB, rz[0:1, N:2 * N])
                nc.vector.tensor_mul(ot[p][0:64, :], raw0, rbcA)
                tmp1 = msc.tile([64, N], BF16, name="tmp1", tag="tmp1", bufs=2)
                nc.vector.tensor_mul(tmp1, raw1, rbcB)
                nc.gpsimd.dma_start(out=ot[p][64:128, :], in_=tmp1)
                if p == PAIRS - 1:
                    for e in range(8):
                        oproj_pending.append((j, e))

            def emit_oproj_chunk():
                j2, e = oproj_pending.pop(0)
                ot = ots[j2]
                py = pp.tile([128, N], F32, name="py", tag="sm", bufs=4)
                for p2 in range(PAIRS):
                    nc.tensor.matmul(py, wo_s[:, p2, 128 * e:128 * (e + 1)],
                                     ot[p2], start=(p2 == 0), stop=(p2 == PAIRS - 1))
                # Pool engine cannot read PSUM; this drain must stay on DVE.
                ys = msc.tile([128, N], F32, name="ys", tag="ys", bufs=4)
                nc.vector.tensor_copy(ys, py)
                oeng = nc.sync if e % 2 == 0 else nc.gpsimd
                oeng.dma_start(
                    out=yt_d[128 * e:128 * (e + 1), N * j2:N * (j2 + 1)], in_=ys)
                if e == 7:
                    del ots[j2]

            # LAG: deep early (scores/exp run ahead while phase-1 PE work
            # drains), shallow later (short pure-PE tail after last exp).
            av_done = 0

            def drain_av(upto):
                nonlocal av_done
                while av_done < upto:
                    j2, p2, g2 = stream[av_done]
                    emit_av(j2, p2, g2)
                    if g2 == NG - 1:
                        emit_norm(j2, p2)
                    av_done += 1

            for idx, (j, p, g) in enumerate(stream):
                for f in side.get(idx, []):
                    f()
                emit_scores_exp(j, p, g)
                lag = (8 if idx < 64 else 4 if idx < 96 else
                       3 if idx < 240 else 2)
                drain_av(idx + 1 - lag)
                if oproj_pending:
                    emit_oproj_chunk()
            drain_av(len(stream))
            while oproj_pending:
                emit_oproj_chunk()

    nc.compile()
    return nc


def _get_nc():
    if "nc" not in _CACHE:
        _CACHE["nc"] = _build()
    return _CACHE["nc"]


def _make_in_maps(inputs):
    import ml_dtypes

    BF = ml_dtypes.bfloat16
    F8 = ml_dtypes.float8_e4m3
    q = np.asarray(inputs["q"], dtype=np.float32)
    k = np.asarray(inputs["k"], dtype=np.float32)
    v = np.asarray(inputs["v"], dtype=np.float32)
    w_q = np.asarray(inputs["w_q"], dtype=np.float32)
    w_k = np.asarray(inputs["w_k"], dtype=np.float32)
    w_v = np.asarray(inputs["w_v"], dtype=np.float32)
    w_o = np.asarray(inputs["w_o"], dtype=np.float32)
    b_q = np.asarray(inputs["b_q"], dtype=np.float32)

    xT = {}
    for b in range(B):
        xT[("q", b)] = np.ascontiguousarray(q[b].T.astype(BF))
        xT[("k", b)] = np.ascontiguousarray(k[b].T.astype(BF))
        xT[("v", b)] = np.ascontiguousarray(v[b].T.astype(BF))

    def wprep(w, sl):
        return np.ascontiguousarray(
            w[:, sl].astype(BF).reshape(CC, 128, N).transpose(1, 0, 2))

    in_maps = []
    for g in range(8):
        b, hg = g // 2, g % 2
        sl = slice(hg * 512, (hg + 1) * 512)
        in_maps.append({
            "xq": xT[("q", b)], "xk": xT[("k", b)], "xv": xT[("v", b)],
            "wq": wprep(w_q, sl),
            "wk": wprep(w_k, sl),
            "wv": wprep(w_v, sl),
            "wo": np.ascontiguousarray(
                w_o[sl, :].astype(BF).reshape(PAIRS, 128, D).transpose(1, 0, 2)),
            "bq": np.ascontiguousarray(b_q[sl].reshape(PAIRS, 128).T),
        })
    return in_maps


def kernel(q, k, v, w_q, b_q, w_k, b_k, w_v, b_v, w_o, b_o):
    nc = _get_nc()
    from concourse.bass_utils import run_bass_kernel_spmd

    b_v = np.asarray(b_v, dtype=np.float32)
    b_o = np.asarray(b_o, dtype=np.float32)
    w_o = np.asarray(w_o, dtype=np.float32)

    in_maps = _make_in_maps(dict(q=q, k=k, v=v, w_q=w_q, b_q=b_q, w_k=w_k,
                                 w_v=w_v, w_o=w_o))
    res = run_bass_kernel_spmd(nc, in_maps, list(range(8)), trace=False)
    outs = [r["yt"] for r in res.results]

    corr = b_v @ w_o + b_o  # [1024]
    y = np.empty((B, S, D), dtype=np.float32)
    for b in range(B):
        y[b] = outs[2 * b].T + outs[2 * b + 1].T + corr
    return y
